# revision 1
# baseline (speedup 1.0000x reference)
"""MemoryAugmentedLayer kernel v2 for 8 trn2 NeuronCores.

Data-parallel over batch B=32768 (4096 rows/core); einsum partial sums
([M,K]+[M,V], bf16) all-reduced between write and read phases.

v2 changes vs baseline:
- x arrives host-transposed and bf16 (xT [D, B_LOC]) — no on-chip x
  transposes or f32->bf16 conversions.
- Write phase is batch-major: write logits z[b,m] = kvT-block.T @ G, so
  exp output needs no per-m transposes; softmax denominator comes from a
  fused DVE tensor_tensor_reduce against a broadcast exp(bwr) row
  (general bwr support), and 1/s is folded into the kv/vv batch-major
  copies.  The einsums then run with kv_s/vv_s stationary and E_w moving
  (N=512), accumulating feat-major dKmT/dVmT in PSUM.
- AllReduce payload in bf16 (0.5 MB vs 1 MB).
- H0 = km.T @ Wrd precomputed once in setup; per-rep H = H0 + dKmT.T @
  Wrd * (1/B) so the post-AR critical path is only red transposes + dH.
  km_new is never materialized; vm stays in [m,v] layout (no transposes
  at setup or post-AR).
"""

import numpy as np
import ml_dtypes

import concourse.bacc as bacc
import concourse.mybir as mybir
import concourse.tile as tile
from concourse import masks
from concourse.bass_utils import run_bass_kernel_spmd

F32 = mybir.dt.float32
F32R = mybir.dt.float32r
BF16 = mybir.dt.bfloat16

B, D, M, K, V = 32768, 256, 1024, 128, 128
N_CORES = 8
B_LOC = B // N_CORES          # 4096 rows per core
CHUNK = 512                   # batch columns processed per chunk
NCH = B_LOC // CHUNK          # 8 chunks
NBT = CHUNK // 128            # 4 batch tiles of 128 per chunk
MT = M // 128                 # 8 tiles of the memory dim
DT = D // 128                 # 2 tiles of the input dim
INV_B = 1.0 / B
BENCH_BUILD_KW = {"zero_bias": False}


def build_nc(repeat=1, zero_bias=False):
    nc = bacc.Bacc("TRN2", target_bir_lowering=False, debug=False,
                   num_devices=N_CORES)

    xT = nc.dram_tensor("xT", [D, B_LOC], BF16, kind="ExternalInput")
    Wk = nc.dram_tensor("Wk", [D, K], F32, kind="ExternalInput")
    Wv = nc.dram_tensor("Wv", [D, V], F32, kind="ExternalInput")
    Wq = nc.dram_tensor("Wq", [D, K], F32, kind="ExternalInput")
    bk = nc.dram_tensor("bk", [K, 1], F32, kind="ExternalInput")
    bv = nc.dram_tensor("bv", [V, 1], F32, kind="ExternalInput")
    bq = nc.dram_tensor("bq", [K, 1], F32, kind="ExternalInput")
    Wwr = nc.dram_tensor("Wwr", [M, M], F32, kind="ExternalInput")
    Wrd = nc.dram_tensor("Wrd", [M, M], F32, kind="ExternalInput")
    bwr = nc.dram_tensor("bwr", [M, 1], F32, kind="ExternalInput")
    brd = nc.dram_tensor("brd", [M, 1], F32, kind="ExternalInput")
    km = nc.dram_tensor("key_memory", [M, K], F32, kind="ExternalInput")
    vm = nc.dram_tensor("value_memory", [M, V], F32, kind="ExternalInput")
    y = nc.dram_tensor("y", [B_LOC, V], F32, kind="ExternalOutput")

    with tile.TileContext(nc) as tc:
        _emit(nc, tc, xT, Wk, Wv, Wq, bk, bv, bq, Wwr, Wrd, bwr, brd, km, vm,
              y, repeat=repeat, zero_bias=zero_bias)
    nc.compile()
    return nc


def _emit(nc, tc, xT, Wk, Wv, Wq, bk, bv, bq, Wwr, Wrd, bwr, brd, km, vm, y,
          repeat=1, zero_bias=False):
    AF = mybir.ActivationFunctionType
    ALU = mybir.AluOpType

    with (
        tc.tile_pool(name="resident", bufs=1) as rp,
        tc.tile_pool(name="stage", bufs=2) as stage,
        tc.tile_pool(name="stream", bufs=2) as sp,
        tc.tile_pool(name="stream1", bufs=1) as sp1,
        tc.tile_pool(name="ps_acc", bufs=1, space="PSUM") as ps_acc,
        tc.tile_pool(name="ps_mm", bufs=2, space="PSUM") as ps_mm,
        tc.tile_pool(name="ps_tr", bufs=1, space="PSUM") as ps_tr,
        tc.tile_pool(name="dram", bufs=1, space="DRAM") as dp,
    ):
        # ---------------- setup: identities, ones, biases ----------------
        ident = rp.tile([128, 128], F32)
        masks.make_identity(nc, ident[:])
        ident_b = rp.tile([128, 128], BF16)
        nc.vector.tensor_copy(ident_b[:], ident[:])
        ident_r = rp.tile([128, 128], F32R)
        nc.vector.tensor_copy(ident_r[:], ident[:])

        ones_f = rp.tile([128, 1], F32)
        nc.gpsimd.memset(ones_f[:], 1.0)
        ones_r = rp.tile([128, 1], F32R)
        nc.vector.tensor_copy(ones_r[:], ones_f[:])
        one1 = rp.tile([1, 1], F32)
        nc.gpsimd.memset(one1[:], 1.0)
        ones_row = rp.tile([1, 128], F32)
        nc.gpsimd.memset(ones_row[:], 1.0)

        # projection weights as lhsT ([d,128] blocks), bf16
        projw_b = rp.tile([128, DT, 3, 128], BF16)
        for j, W in enumerate((Wk, Wv, Wq)):
            for dt_ in range(DT):
                wst = stage.tile([128, 128], F32, tag="wst")
                nc.sync.dma_start(wst[:], W[dt_ * 128:(dt_ + 1) * 128, :])
                nc.vector.tensor_copy(projw_b[:, dt_, j, :], wst[:])

        bias_p = rp.tile([128, 3], F32)
        for j, b in enumerate((bk, bv, bq)):
            nc.sync.dma_start(bias_p[:, j:j + 1], b[:])
        bias_pm1 = rp.tile([128, 3], F32)
        nc.vector.tensor_scalar_add(bias_pm1[:], bias_p[:], -1.0)
        bias_rd = rp.tile([128, MT], F32)
        for mp in range(MT):
            nc.sync.dma_start(bias_rd[:, mp:mp + 1], brd[mp * 128:(mp + 1) * 128, :])
        # ebr = exp(brd) per-partition; folded into value memory / ones so
        # the phase-2 exp needs no bias (allows batched bias-free ACT ops)
        ebr_cols = rp.tile([128, MT], F32)
        nc.scalar.activation(ebr_cols[:], bias_rd[:], AF.Exp)
        ebr_r = rp.tile([128, MT], F32R)
        nc.vector.tensor_copy(ebr_r[:], ebr_cols[:])
        ebr_inv = rp.tile([128, MT], F32)
        nc.vector.tensor_scalar_mul(ebr_inv[:], ebr_cols[:], INV_B)

        # ebw_bcast[p, m] = exp(bwr[m]) replicated across partitions
        # (general-bwr path only; the zero-bias build folds nothing)
        ebw_bcast = None
        if not zero_bias:
            bwr_row = rp.tile([1, M], F32)
            nc.sync.dma_start(bwr_row[:],
                              bwr.rearrange("(o m) x -> o (m x)", o=1))
            ebw_bcast = rp.tile([128, M], BF16)
            for half in range(2):
                pbw = ps_mm.tile([128, 512], F32, tag="mm")
                nc.tensor.matmul(pbw[:], ones_row[:],
                                 bwr_row[:, half * 512:(half + 1) * 512],
                                 start=True, stop=True)
                nc.scalar.activation(ebw_bcast[:, half * 512:(half + 1) * 512],
                                     pbw[:], AF.Exp)

        # ---- G = key_memory.T @ Wwr (bf16) ----
        g_lo = ps_acc.tile([128, 512], F32, tag="acc_k_lo")
        g_hi = ps_acc.tile([128, 512], F32, tag="acc_k_hi")
        for mk in range(MT):
            mst = stage.tile([128, 128], F32, tag="mst")
            nc.sync.dma_start(mst[:], km[mk * 128:(mk + 1) * 128, :])
            km_b = stage.tile([128, 128], BF16, tag="km_b")
            nc.vector.tensor_copy(km_b[:], mst[:])
            wwrt = stage.tile([128, M], F32, tag="wbig")
            nc.sync.dma_start(wwrt[:], Wwr[mk * 128:(mk + 1) * 128, :])
            wwrt_b = stage.tile([128, M], BF16, tag="wbig_b")
            nc.vector.tensor_copy(wwrt_b[:], wwrt[:])
            nc.tensor.matmul(g_lo[:], km_b[:], wwrt_b[:, 0:512],
                             start=(mk == 0), stop=(mk == MT - 1),
                             skip_group_check=True)
            nc.tensor.matmul(g_hi[:], km_b[:], wwrt_b[:, 512:M],
                             start=(mk == 0), stop=(mk == MT - 1),
                             skip_group_check=True)
        G_b = rp.tile([128, M], BF16)
        nc.vector.tensor_copy(G_b[:, 0:512], g_lo[:])
        nc.vector.tensor_copy(G_b[:, 512:M], g_hi[:])

        # ---- km blocks (f32r, [m,k] layout straight from DRAM) ----
        km_r = rp.tile([128, MT, 128], F32R)
        for mk in range(MT):
            mst2 = stage.tile([128, 128], F32, tag="mst")
            nc.sync.dma_start(mst2[:], km[mk * 128:(mk + 1) * 128, :])
            nc.vector.tensor_copy(km_r[:, mk, :], mst2[:])

        # ---- vm blocks ([m,v] layout from DRAM), pre-scaled by ebr ----
        vm_sb = rp.tile([128, MT, 128], F32)
        for mk in range(MT):
            vmst = stage.tile([128, 128], F32, tag="mst")
            nc.sync.dma_start(vmst[:], vm[mk * 128:(mk + 1) * 128, :])
            nc.vector.tensor_scalar_mul(vm_sb[:, mk, :], vmst[:],
                                        ebr_cols[:, mk:mk + 1])

        # ---- Wrd resident bf16 (for per-rep dH) + H0 = km.T @ Wrd ----
        wrd_rb = [rp.tile([128, M], BF16, name=f"wrd_rb{i}") for i in range(MT)]
        h_lo = ps_mm.tile([128, 512], F32, tag="mm")
        h_hi = ps_mm.tile([128, 512], F32, tag="mm")
        for mk in range(MT):
            wst3 = stage.tile([128, M], F32, tag="wbig")
            nc.sync.dma_start(wst3[:], Wrd[mk * 128:(mk + 1) * 128, :])
            nc.vector.tensor_copy(wrd_rb[mk][:], wst3[:])
            wst3r = stage.tile([128, M], F32R, tag="wbig_r")
            nc.vector.tensor_copy(wst3r[:], wst3[:])
            nc.tensor.matmul(h_lo[:], km_r[:, mk, :], wst3r[:, 0:512],
                             start=(mk == 0), stop=(mk == MT - 1),
                             skip_group_check=True)
            nc.tensor.matmul(h_hi[:], km_r[:, mk, :], wst3r[:, 512:M],
                             start=(mk == 0), stop=(mk == MT - 1),
                             skip_group_check=True)
        H0_sb = rp.tile([128, M], F32)
        nc.vector.tensor_copy(H0_sb[:, 0:512], h_lo[:])
        nc.vector.tensor_copy(H0_sb[:, 512:M], h_hi[:])

        # qry kept for phase 2; double-buffered so rep r+1's phase 1 can
        # overlap rep r's collective + phase 2
        qryT_bufs = [rp.tile([128, B_LOC], F32R, name=f"qryT{i}")
                     for i in range(2)]

        # Software-pipelined rep loop: engine queues execute in emission
        # order, so rep r's first F chunk front-halves are EMITTED before
        # rep r-1's post-AR + phase 2 — they fill the AllReduce window.
        xT_tiled = xT.rearrange("(dt p) (h c) -> h p dt c", p=128, c=CHUNK)
        y_tiled = y.rearrange("(h t p) v -> h p t v", p=128, t=NBT)
        fargs = (nc, sp, sp1, ps_mm, ps_tr, ident_b, projw_b, bias_p,
                 bias_pm1, ebw_bcast, G_b, xT_tiled)
        targs = (nc, rp, sp, ps_acc, ps_mm, ps_tr, ident_r, ident_b, ones_r,
                 one1, ebr_r, ebr_inv, wrd_rb, H0_sb, vm_sb, y_tiled)
        F = 3
        prev = None
        for _rep in range(repeat):
            qry = qryT_bufs[_rep % 2]
            backlog = []
            if prev is not None:
                for h in range(F):
                    backlog.append(_emit_front(*fargs, qry, h))
                _emit_tail(*targs, *prev)
            accs = (ps_acc.tile([128, 512], F32, tag="acc_k_lo",
                                name=f"pk_lo{_rep}"),
                    ps_acc.tile([128, 512], F32, tag="acc_k_hi",
                                name=f"pk_hi{_rep}"),
                    ps_acc.tile([128, 512], F32, tag="acc_v_lo",
                                name=f"pv_lo{_rep}"),
                    ps_acc.tile([128, 512], F32, tag="acc_v_hi",
                                name=f"pv_hi{_rep}"))
            carry = None
            first = True
            for h in range(NCH):
                cur = backlog[h] if h < len(backlog) else                     _emit_front(*fargs, qry, h)
                if carry is not None:
                    _emit_einsum(nc, accs, carry, first=first, last=False)
                    first = False
                carry = cur
            _emit_einsum(nc, accs, carry, first=first, last=True)
            red_sb = _emit_ar(nc, rp, dp, accs, _rep)
            prev = (red_sb, qry)
        _emit_tail(*targs, *prev)


def _emit_einsum(nc, accs, carry, first, last):
    kv_s, vv_s, E_w = carry
    pk_lo, pk_hi, pv_lo, pv_hi = accs
    for t in range(NBT):
        f = first and t == 0
        l = last and t == NBT - 1
        nc.tensor.matmul(pk_lo[:], kv_s[:, t, :], E_w[:, t, 0:512],
                         start=f, stop=l, skip_group_check=True)
        nc.tensor.matmul(pk_hi[:], kv_s[:, t, :], E_w[:, t, 512:M],
                         start=f, stop=l, skip_group_check=True)
        nc.tensor.matmul(pv_lo[:], vv_s[:, t, :], E_w[:, t, 0:512],
                         start=f, stop=l, skip_group_check=True)
        nc.tensor.matmul(pv_hi[:], vv_s[:, t, :], E_w[:, t, 512:M],
                         start=f, stop=l, skip_group_check=True)


def _emit_front(nc, sp, sp1, ps_mm, ps_tr, ident_b, projw_b, bias_p,
                bias_pm1, ebw_bcast, G_b, xT_tiled, qryT_r, h):
    """Chunk front-half: projections+elu, write logits+exp+row-sums,
    kv/vv batch-major scaled copies. Returns the einsum carry."""
    AF = mybir.ActivationFunctionType
    ALU = mybir.AluOpType

    xTc = sp.tile([128, DT, CHUNK], BF16, tag="xTc", bufs=3)
    nc.sync.dma_start(xTc[:], xT_tiled[h])

    kvT = sp.tile([128, CHUNK], BF16, tag="kvT", bufs=3)
    vvT = sp.tile([128, CHUNK], BF16, tag="vvT", bufs=3)
    for j in range(3):
        pp = ps_mm.tile([128, CHUNK], F32, tag="mm")
        for dt_ in range(DT):
            nc.tensor.matmul(pp[:], projw_b[:, dt_, j, :], xTc[:, dt_, :],
                             start=(dt_ == 0), stop=(dt_ == DT - 1))
        # elu(z+b) = [max(z+b-1, -1)] + [min(exp(z+b), 1)]
        edt = F32 if j == 2 else BF16
        texp = sp.tile([128, CHUNK], edt, tag=f"texp{j == 2}", bufs=2)
        nc.scalar.activation(texp[:], pp[:], AF.Exp, bias=bias_p[:, j:j + 1])
        trelu = sp.tile([128, CHUNK], edt, tag=f"trelu{j == 2}", bufs=2)
        nc.vector.tensor_scalar(out=trelu[:], in0=pp[:],
                                scalar1=bias_pm1[:, j:j + 1],
                                scalar2=-1.0, op0=ALU.add, op1=ALU.max)
        dst = (kvT[:], vvT[:], qryT_r[:, h * CHUNK:(h + 1) * CHUNK])[j]
        nc.vector.scalar_tensor_tensor(dst, texp[:], 1.0, trelu[:],
                                       ALU.min, ALU.add)

    E_w = sp1.tile([128, NBT, M], BF16, tag="E_w", bufs=4)
    s_col = sp.tile([128, NBT], F32, tag="s_col", bufs=2)
    s_hA = sp.tile([128, NBT], F32, tag="s_hA", bufs=2)
    s_hB = sp.tile([128, NBT], F32, tag="s_hB", bufs=2)
    for t in range(NBT):
        for half in range(2):
            pz = ps_mm.tile([128, 512], F32, tag="mm")
            nc.tensor.matmul(pz[:], kvT[:, t * 128:(t + 1) * 128],
                             G_b[:, half * 512:(half + 1) * 512],
                             start=True, stop=True)
            sdst = (s_hA if half == 0 else s_hB)[:, t:t + 1]
            if ebw_bcast is None:
                nc.scalar.activation(
                    E_w[:, t, half * 512:(half + 1) * 512], pz[:], AF.Exp,
                    accum_out=sdst)
            else:
                eraw = sp.tile([128, 512], BF16, tag="eraw", bufs=3)
                nc.scalar.activation(eraw[:], pz[:], AF.Exp)
                nc.vector.scalar_tensor_tensor(
                    E_w[:, t, half * 512:(half + 1) * 512], eraw[:], 1.0,
                    ebw_bcast[:, half * 512:(half + 1) * 512],
                    ALU.mult, ALU.mult, accum_out=sdst)
    rw = sp.tile([128, NBT], F32, tag="rw", bufs=2)
    nc.vector.tensor_tensor(s_col[:], s_hA[:], s_hB[:], ALU.add)
    nc.vector.reciprocal(rw[:], s_col[:])

    kv_s = sp.tile([128, NBT, 128], BF16, tag="kv_s", bufs=5)
    vv_s = sp.tile([128, NBT, 128], BF16, tag="vv_s", bufs=5)
    for src_, dstt in ((kvT, kv_s), (vvT, vv_s)):
        ptk = ps_tr.tile([128, NBT, 128], BF16, tag="trb", bufs=2)
        for t in range(NBT):
            nc.tensor.matmul(ptk[:, t, :], src_[:, t * 128:(t + 1) * 128],
                             ident_b[:], is_transpose=True,
                             start=True, stop=True, skip_group_check=True)
        for t in range(NBT):
            nc.vector.tensor_scalar_mul(dstt[:, t, :], ptk[:, t, :],
                                        rw[:, t:t + 1])
    return kv_s, vv_s, E_w


def _emit_ar(nc, rp, dp, accs, rep_i):
    pk_lo, pk_hi, pv_lo, pv_hi = accs
    part_sb = rp.tile([128, 2 * M], BF16, tag="part_sb", name=f"part{rep_i}")
    nc.vector.tensor_copy(part_sb[:, 0:512], pk_lo[:])
    nc.scalar.copy(part_sb[:, 512:1024], pk_hi[:])
    nc.vector.tensor_copy(part_sb[:, 1024:1536], pv_lo[:])
    nc.scalar.copy(part_sb[:, 1536:2048], pv_hi[:])
    cc_in = dp.tile([128, 2 * M], BF16, tag="cc_in", name=f"cc_in{rep_i}")
    cc_out = dp.tile([128, 2 * M], BF16, tag="cc_out", name=f"cc_out{rep_i}")
    nc.sync.dma_start(cc_in[:], part_sb[:])
    nc.gpsimd.collective_compute(
        "AllReduce", mybir.AluOpType.add,
        replica_groups=[list(range(N_CORES))],
        ins=[cc_in.opt()], outs=[cc_out.opt()],
    )
    red_sb = rp.tile([128, 2 * M], BF16, tag="red_sb", name=f"red{rep_i}")
    nc.sync.dma_start(red_sb[:], cc_out[:])
    return red_sb


def _emit_tail(nc, rp, sp, ps_acc, ps_mm, ps_tr, ident_r, ident_b, ones_r,
               one1, ebr_r, ebr_inv, wrd_rb, H0_sb, vm_sb, y_tiled,
               red_sb, qryT_r):
    """Post-AR (transposes + dH + vm update + H) and phase 2."""
    AF = mybir.ActivationFunctionType
    ALU = mybir.AluOpType

    rkm = sp.tile([128, MT, 128], BF16, tag="rkm")
    vmn_r = rp.tile([128, MT, 128], F32R, tag="vmn_r")
    dh_lo = ps_acc.tile([128, 512], F32, tag="acc_k_lo")
    dh_hi = ps_acc.tile([128, 512], F32, tag="acc_k_hi")
    for mk in range(MT):
        ptm = ps_tr.tile([128, 128], BF16, tag="trb", bufs=2)
        nc.tensor.matmul(ptm[:], red_sb[:, mk * 128:(mk + 1) * 128],
                         ident_b[:], is_transpose=True, start=True, stop=True)
        nc.vector.tensor_copy(rkm[:, mk, :], ptm[:])
        nc.tensor.matmul(dh_lo[:], rkm[:, mk, :], wrd_rb[mk][:, 0:512],
                         start=(mk == 0), stop=(mk == MT - 1),
                         skip_group_check=True)
        nc.tensor.matmul(dh_hi[:], rkm[:, mk, :], wrd_rb[mk][:, 512:M],
                         start=(mk == 0), stop=(mk == MT - 1),
                         skip_group_check=True)
        ptv = ps_tr.tile([128, 128], BF16, tag="trb", bufs=2)
        nc.tensor.matmul(ptv[:], red_sb[:, M + mk * 128:M + (mk + 1) * 128],
                         ident_b[:], is_transpose=True, start=True, stop=True)
        nc.vector.scalar_tensor_tensor(vmn_r[:, mk, :], ptv[:],
                                       ebr_inv[:, mk:mk + 1],
                                       vm_sb[:, mk, :], ALU.mult, ALU.add)
    H_r = rp.tile([128, M], F32R, tag="H_r")
    nc.vector.scalar_tensor_tensor(H_r[:, 0:512], dh_lo[:], INV_B,
                                   H0_sb[:, 0:512], ALU.mult, ALU.add)
    nc.vector.scalar_tensor_tensor(H_r[:, 512:M], dh_hi[:], INV_B,
                                   H0_sb[:, 512:M], ALU.mult, ALU.add)

    # phase 2: chunk pairs (two interleaved chains share stationaries)
    for hp in range(NCH // 2):
        hs = (2 * hp, 2 * hp + 1)
        u_tags = ("acc_k_lo", "acc_v_lo")
        s_tags = ("acc_k_hi", "acc_v_hi")
        u_pss = [ps_acc.tile([128, CHUNK], F32, tag=u_tags[i], name=f"u{i}")
                 for i in range(2)]
        s_pss = [ps_acc.tile([1, CHUNK], F32, tag=s_tags[i], name=f"s{i}")
                 for i in range(2)]
        for mp in range(MT):
            for i, h in enumerate(hs):
                qslice = qryT_r[:, h * CHUNK:(h + 1) * CHUNK]
                prl = ps_mm.tile([128, CHUNK], F32, tag="mm")
                nc.tensor.matmul(prl[:], H_r[:, mp * 128:(mp + 1) * 128],
                                 qslice, start=True, stop=True)
                erT = sp.tile([128, CHUNK], F32R, tag="erT", bufs=4)
                nc.scalar.activation(erT[:], prl[:], AF.Exp)
                nc.tensor.matmul(u_pss[i][:], vmn_r[:, mp, :], erT[:],
                                 start=(mp == 0), stop=(mp == MT - 1),
                                 skip_group_check=True)
                nc.tensor.matmul(s_pss[i][:], ebr_r[:, mp:mp + 1], erT[:],
                                 start=(mp == 0), stop=(mp == MT - 1),
                                 skip_group_check=True)

        for i, h in enumerate(hs):
            u_ps, s_ps = u_pss[i], s_pss[i]
            s_sb = sp.tile([1, CHUNK], F32, tag="s_sb")
            nc.vector.tensor_copy(s_sb[:], s_ps[:])
            s_cols = sp.tile([128, NBT], F32, tag="s_cols")
            for t in range(NBT):
                pst = ps_mm.tile([128, 1], F32, tag="mm")
                nc.tensor.matmul(pst[:], s_sb[0:1, t * 128:(t + 1) * 128],
                                 one1[:], start=True, stop=True)
                nc.vector.tensor_copy(s_cols[:, t:t + 1], pst[:])
            r_cols = sp.tile([128, NBT], F32, tag="r_cols")
            nc.vector.reciprocal(r_cols[:], s_cols[:])

            u_sb = sp.tile([128, CHUNK], F32R, tag="u_sb")
            nc.vector.tensor_copy(u_sb[:], u_ps[:])
            ot = sp.tile([128, NBT, V], F32, tag="ot", bufs=2)
            for t in range(NBT):
                ptu = ps_tr.tile([128, 128], F32R, tag="trb", bufs=2)
                nc.tensor.matmul(ptu[:], u_sb[:, t * 128:(t + 1) * 128],
                                 ident_r[:], is_transpose=True,
                                 start=True, stop=True)
                nc.vector.tensor_scalar_mul(ot[:, t, :], ptu[:],
                                            r_cols[:, t:t + 1])
            nc.sync.dma_start(y_tiled[h], ot[:])


_NC_CACHE = {}


def _get_nc(zero_bias=False):
    if zero_bias not in _NC_CACHE:
        _NC_CACHE[zero_bias] = build_nc(zero_bias=zero_bias)
    return _NC_CACHE[zero_bias]


def make_in_maps(inputs):
    xs = np.ascontiguousarray(np.asarray(inputs["x"], dtype=np.float32))
    rep = {}
    for name in ("Wk", "Wv", "Wq", "Wwr", "Wrd", "key_memory", "value_memory"):
        rep[name] = np.ascontiguousarray(np.asarray(inputs[name], np.float32))
    for name in ("bk", "bv", "bq", "bwr", "brd"):
        rep[name] = np.ascontiguousarray(
            np.asarray(inputs[name], np.float32).reshape(-1, 1))
    in_maps = []
    for c in range(N_CORES):
        xc = xs[c * B_LOC:(c + 1) * B_LOC]
        m = {"xT": np.ascontiguousarray(xc.T).astype(ml_dtypes.bfloat16)}
        m.update(rep)
        in_maps.append(m)
    return in_maps


def kernel(**inputs):
    nc = _get_nc(zero_bias=False)
    in_maps = make_in_maps(inputs)
    res = run_bass_kernel_spmd(nc, in_maps, core_ids=list(range(N_CORES)))
    return np.concatenate([r["y"] for r in res.results], axis=0)

